# revision 22
# baseline (speedup 1.0000x reference)
"""AdaptiveBoundaryRankingLoss on 8 TRN2 NeuronCores — band algorithm.

loss = (1/K) sum_{pairs} relu(B(|dt|) - (p_hi - p_lo)),
  B(a) = BETA*a/(1+GAMMA*a), K = B(B-1)/2, hi = larger-target index.

Host sorts by PRED ascending. For i > j (dp = p_i - p_j >= 0):
  - discordant pairs (t_i < t_j): contribution = B(|dt|) + dp, relu-free.
    Computed EXACTLY on host in O(n log n) via a weighted merge pass
    (per-i sums of t_j^a over inversions) + the power series of B.
  - concordant pairs (t_i > t_j): relu(B(dt) - dp), nonzero only when
    dp < max B ~ 0.273 -> a narrow band near the diagonal (~5M of 33.5M
    pairs). A global quadratic q(u) ~ B(u) on [0, L] with q(0) <= 0 and
    q concave zeroes discordant band pairs automatically (q(u<0) < 0 <= dp),
    so the band term is relu of a rank-4 bilinear form:
      z_ij = bias_i + ct_i*t_j + c2*t_j^2 + p_j,
      bias_i = c0 + c1 t_i + c2 t_i^2 - p_i, ct_i = -c1 - 2 c2 t_i.
    The within-block diagonal triangles (1.5% of pairs; z host-computable
    exactly) are folded into the host term.

Device (per core, SPMD): TensorE materializes z for 256-col chunks via
[7,128]^T @ [7,256] bf16 matmuls into PSUM (hi/lo-split coefficients for
precision); ScalarE (Relu activation, accum_out) and VectorE
(tensor_scalar max-then-add, accum_out) relu+row-sum alternating
1024-col PSUM groups (4 two-bank buffers). A dummy-matmul burst warms
the PE HAM clock gate during the input DMA. Per-group [128,1] partial
sums land in one acc table, DMA'd out once; host reduces in f64.
"""

import contextlib
import math

import numpy as np
import ml_dtypes

import concourse.bass as bass
from concourse import mybir
from concourse.bass_utils import run_bass_kernel_spmd

B = 8192
BETA = 0.3
GAMMA = 0.1
NCORES = 8
P = 128
CH = 256          # matmul chunk width (cols)
CPG = 4           # chunks per relu group (group = 1024 PSUM cols = 2 banks)
NBLK = B // P     # 64 row blocks
NDUM = 30         # PE warmup dummy matmuls

_bf16 = ml_dtypes.bfloat16

_NC_CACHE = {}


def _Bfun(a):
    return BETA * a / (1.0 + GAMMA * a)


# ---------- host: exact discordant closed form ----------

def _disc_sums(t, p, M):
    """S[i, a] = sum_{j<i, t_j > t_i} t_j^a (a=0..M); S[i, M+1] same for p_j.
    Bottom-up merge, O(n log n). n must be a power of two."""
    n = len(t)
    W = np.empty((n, M + 2))
    W[:, 0] = 1.0
    for a in range(1, M + 1):
        W[:, a] = W[:, a - 1] * t
    W[:, M + 1] = p
    S = np.zeros((n, M + 2))
    idx = np.arange(n)
    L = 1
    while L < n:
        nruns = n // (2 * L)
        run = idx.reshape(nruns, 2, L)
        li, ri = run[:, 0, :], run[:, 1, :]
        if L <= 64:
            mask = t[li][:, :, None] > t[ri][:, None, :]
            contrib = np.einsum('pji,pjw->piw', mask, W[li])
            S[ri.ravel()] += contrib.reshape(-1, M + 2)
        else:
            for k in range(nruns):
                tl = t[li[k]]
                pos = np.searchsorted(tl, t[ri[k]], side='right')
                suf = np.vstack([np.cumsum(W[li[k]][::-1], axis=0)[::-1],
                                 np.zeros((1, M + 2))])
                S[ri[k]] += suf[pos]
        tv = t[idx].reshape(nruns, 2 * L)
        ordr = np.argsort(tv, axis=1, kind='stable')
        idx = np.take_along_axis(idx.reshape(nruns, 2 * L), ordr, axis=1).ravel()
        L *= 2
    return S


def _disc_closed_form(t, p, M=18):
    """sum over discordant pairs (i>j in p-order, t_j > t_i) of
    B(t_j - t_i) + (p_i - p_j), exact (B via power series)."""
    n = len(t)
    if n & (n - 1) != 0 or (GAMMA * (t.max() - t.min())) > 0.5:
        # fallback: chunked brute force in f64
        tb = 0.0
        for s in range(0, n, 512):
            e = min(s + 512, n)
            u = t[s:e, None] - t[None, :]
            dp = p[s:e, None] - p[None, :]
            lower = (np.arange(s, e)[:, None] > np.arange(n)[None, :])
            disc = lower & (u < 0)
            tb += (_Bfun(-u[disc]) + dp[disc]).sum()
        return tb
    S = _disc_sums(t, p, M)
    total = float((p * S[:, 0]).sum() - S[:, M + 1].sum())
    negt_pow = np.empty((n, M + 1))
    negt_pow[:, 0] = 1.0
    for b in range(1, M + 1):
        negt_pow[:, b] = negt_pow[:, b - 1] * (-t)
    for m in range(1, M + 1):
        Tm = 0.0
        for a in range(0, m + 1):
            Tm += math.comb(m, a) * float((S[:, a] * negt_pow[:, m - a]).sum())
        total += BETA * ((-GAMMA) ** (m - 1)) * Tm
    return total


# ---------- host: quadratic fit of B on [0, L] ----------

def _quad_fit(L):
    x = np.linspace(0.0, L, 8001)
    y = _Bfun(x)
    A = np.stack([np.ones_like(x), x, x * x], 1)
    wts = np.ones_like(x)
    c = np.zeros(3)
    for _ in range(40):
        c = np.linalg.lstsq(A * wts[:, None], y * wts, rcond=None)[0]
        r = np.abs(A @ c - y)
        wts *= (1e-12 + r) ** 0.5
        wts /= wts.max()
    # pin c2 to an exact bf16 value, refit c0, c1
    c2 = float(np.float64(_bf16(c[2])))
    y2 = y - c2 * x * x
    A2 = A[:, :2]
    wts = np.ones_like(x)
    for _ in range(40):
        c01 = np.linalg.lstsq(A2 * wts[:, None], y2 * wts, rcond=None)[0]
        r = np.abs(A2 @ c01 - y2)
        wts *= (1e-12 + r) ** 0.5
        wts /= wts.max()
    c0, c1 = float(c01[0]), float(c01[1])
    resid = float(np.abs(c0 + c1 * x + c2 * x * x - y).max())
    if c0 > 0:
        c0 = -1e-6
    assert c1 > 0 and c2 < 0
    return c0, c1, c2, resid


# ---------- bass graph ----------

def _group_plan(NCH):
    """Plan relu groups for NCH chunks: sizes (in 256-col chunks), engine
    per group, and per-chunk (lane, g3) table coordinates plus the PE
    processing order. Within a 4-chunk group the PE visits banks
    alternately on two different partition lanes so LDWEIGHTS of the next
    matmul overlaps the current one (different PE row strips)."""
    szs = [2]
    rem = NCH - 2
    tail = [2, 2]
    while rem - sum(tail) >= 4:
        szs.append(4)
        rem -= 4
    for t_ in tail:
        if rem >= t_:
            szs.append(t_)
            rem -= t_
    while rem > 0:
        szs.append(min(4, rem))
        rem -= min(4, rem)
    NG = len(szs)
    # engine per group: greedy balance of estimated busy time
    tS = 1.0   # head start penalty none; costs in ns
    tV = 0.0
    eng = []
    for sz in szs:
        cS = (172 + sz * 256) / 1.2 + 285
        cV = (120 + sz * 256) / 0.96 + 83
        if tS + cS <= tV + cV:
            eng.append('S'); tS += cS
        else:
            eng.append('V'); tV += cV
    # lanes: 4-chunk groups use a pair (la for cols 0-1, lb for cols 2-3);
    # 2-chunk groups one lane. Greedy balance chunks per lane.
    loads = [0, 0, 0]
    lane_of = []      # per data chunk (column order)
    g3_of = []
    order = []        # per group: col indices in PE processing order
    for sz in szs:
        la = min(range(3), key=lambda i: loads[i])
        lanes = [la] * sz
        order.append(list(range(sz)))
        for l in lanes:
            lane_of.append(l)
            g3_of.append(loads[l])
            loads[l] += 1
    G3L = max(loads)
    return szs, eng, lane_of, g3_of, order, G3L


def build_nc(NCH):
    # NCH must be a multiple of 12 (3 partition lanes x 4 chunks/group pad)
    nc = bass.Bass(target_bir_lowering=False, debug=False)
    f32 = mybir.dt.float32
    bf = mybir.dt.bfloat16
    GW = CPG * CH
    CB = CH + P           # per-chunk table block: 256 colv + 128 stat cols
    G3 = NCH // 3         # table column groups (3 lanes at partitions 0/32/64)
    Relu = mybir.ActivationFunctionType.Relu
    A = mybir.AluOpType

    SZ, ENG, chunk_lane, chunk_g3, ORDER, G3L = _group_plan(NCH)
    NG = len(SZ)
    NGS = ENG.count('S')
    NGV = ENG.count('V')
    # DMA quarters: small first quarter so the PE can start early
    rest = max(1, (G3L - 1 + 2) // 3)
    qb = [0, 1]
    for q in range(3):
        qb.append(min(G3L, qb[-1] + rest))
    qb[4] = G3L

    tbl_d = nc.declare_dram_parameter("tbl", [96, G3L * CB], bf, isOutput=False)
    out_d = nc.declare_dram_parameter("acc", [P, NG], f32, isOutput=True)

    es = contextlib.ExitStack()
    with es:
        def sb(name, shape, dtype):
            return es.enter_context(nc.sbuf_tensor(name, shape, dtype))

        tbl = sb("tbl_s", [96, G3L * CB], bf)
        junk = sb("junk", [7, 64], bf)
        wS = sb("wS", [P, GW], bf)
        wV = sb("wV", [P, GW], bf)
        acc = sb("acc_s", [P, NG], f32)
        psA = es.enter_context(nc.psum_tensor("psA", [P, 2 * GW], f32))
        psB = es.enter_context(nc.psum_tensor("psB", [P, 2 * GW], f32))
        dq = [es.enter_context(nc.semaphore(f"dq{q}")) for q in range(4)]
        te_s = es.enter_context(nc.semaphore("te_s"))
        sS = es.enter_context(nc.semaphore("sS"))
        sV = es.enter_context(nc.semaphore("sV"))

        # pre-block quarter DMAs on the sync queue (q0, q2); the scalar
        # engine issues q1/q3 inside its block body after the ACT table load
        def issue_quarter(eng, q):
            sl = slice(qb[q] * CB, qb[q + 1] * CB)
            if sl.start < sl.stop:
                eng.dma_start(out=tbl[:, sl], in_=tbl_d[:, sl]).then_inc(dq[q], 16)
            else:
                eng.memset(junk[0:1, 0:1], 0.0).then_inc(dq[q], 16)

        block = es.enter_context(nc.Block())

        def slot_of(g):
            ps = psA if (g % 4) < 2 else psB
            off = (g % 2) * GW
            return ps, off

        @block.sync
        def _(sync):
            issue_quarter(sync, 0)
            issue_quarter(sync, 2)
            sync.wait_ge(sS, NGS)
            sync.wait_ge(sV, NGV)
            sync.dma_start(out=out_d[:, :], in_=acc[:, :]).then_inc(dq[0], 16)

        @block.tensor
        def _(tensor):
            # dummy burst: keeps PE busy through the DMA window so the HAM
            # activity window opens as early as possible
            for _ in range(NDUM):
                tensor.matmul(psB[0:64, 0:64], junk[:, :], junk[:, :],
                              start=True, stop=True)
            qdone = -1
            cum = 0
            for g, sz in enumerate(SZ):
                ps, off = slot_of(g)
                dep = g - 4
                if dep >= 0:
                    eng = ENG[dep]
                    idx = sum(1 for x in ENG[:dep + 1] if x == eng)
                    tensor.wait_ge(sV if eng == 'V' else sS, idx)
                mm = None
                for k in ORDER[g]:
                    d = cum + k
                    lane = chunk_lane[d]
                    g3 = chunk_g3[d]
                    qneed = next(q for q in range(4) if g3 < qb[q + 1])
                    if qneed > qdone:
                        for q2 in range(qdone + 1, qneed + 1):
                            tensor.wait_ge(dq[q2], 16)
                        qdone = qneed
                    base = g3 * CB
                    mm = tensor.matmul(
                        ps[:, off + k * CH:off + (k + 1) * CH],
                        tbl[32 * lane:32 * lane + 7, base + CH:base + CB],
                        tbl[32 * lane:32 * lane + 7, base:base + CH],
                        start=True, stop=True,
                    )
                cum += sz
                mm.then_inc(te_s, 1)

        @block.scalar
        def _(scalar):
            # tiny dummy activation first: pulls ACT_TABLE_LOAD to the very
            # start of the scalar stream (no DMA issue ahead of it now)
            scalar.activation(wS[:, 0:1], wS[:, 0:1], Relu)
            issue_quarter(scalar, 1)
            issue_quarter(scalar, 3)
            ns = 0
            for g, sz in enumerate(SZ):
                if ENG[g] != 'S':
                    continue
                ps, off = slot_of(g)
                ns += 1
                scalar.wait_ge(te_s, g + 1)
                scalar.activation(
                    wS[:, :sz * CH], ps[:, off:off + sz * CH], Relu,
                    bias=0.0, scale=1.0,
                    accum_out=acc[:, g:g + 1],
                ).then_inc(sS, 1)


        @block.vector
        def _(vector):
            for g, sz in enumerate(SZ):
                if ENG[g] != 'V':
                    continue
                ps, off = slot_of(g)
                vector.wait_ge(te_s, g + 1)
                vector.tensor_scalar(
                    out=wV[:, :sz * CH], in0=ps[:, off:off + sz * CH],
                    scalar1=0.0, scalar2=0.0, op0=A.max, op1=A.add,
                    accum_out=acc[:, g:g + 1],
                ).then_inc(sV, 1)

    return nc


def _get_nc(NCH):
    if NCH not in _NC_CACHE:
        _NC_CACHE[NCH] = build_nc(NCH)
    return _NC_CACHE[NCH]


# ---------- host: layout + input baking ----------

def _prepare(pred, target):
    p64 = np.asarray(pred, np.float64)
    t64 = np.asarray(target, np.float64)
    n = len(p64)
    order = np.argsort(p64, kind="stable")
    p = p64[order]
    t = t64[order]

    host_total = _disc_closed_form(t, p)

    Lspan = float(t.max() - t.min())
    Lspan = max(Lspan, 1e-6)
    c0, c1, c2, resid = _quad_fit(Lspan)
    qmax = max(_Bfun(Lspan), c0 + c1 * Lspan + c2 * Lspan * Lspan)
    DPMAX = qmax + 2 * resid + 1e-6

    # diagonal 128x128 triangles: exact host relu-sum (z is host-known)
    tb = t.reshape(NBLK, P)
    pb = p.reshape(NBLK, P)
    u = tb[:, :, None] - tb[:, None, :]
    dpd = pb[:, :, None] - pb[:, None, :]
    zd = c0 + c1 * u + c2 * u * u - dpd
    m = np.tril(np.ones((P, P), bool), -1)[None, :, :]
    host_total += float(np.where(m, np.maximum(zd, 0.0), 0.0).sum())

    lo = np.searchsorted(p, p - DPMAX, side="left")

    nch_b = []
    for b in range(NBLK):
        r0 = P * b
        span = r0 - int(lo[r0])
        nch_b.append((span + CH - 1) // CH)

    # greedy balance blocks' main chunks over cores
    loads = [0] * NCORES
    assign = [[] for _ in range(NCORES)]
    for b in sorted(range(NBLK), key=lambda b: -nch_b[b]):
        c = min(range(NCORES), key=lambda c: loads[c])
        loads[c] += nch_b[b]
        assign[c].append(b)
    NCH = max(1, -(-max(loads) // 12)) * 12  # 3 lanes x 4 DMA quarters

    # per-row quantities (f64 -> f32 -> bf16 hi/lo)
    bias = (c0 + c1 * t + c2 * t * t - p).astype(np.float32).astype(np.float64)
    ct = (-c1 - 2.0 * c2 * t).astype(np.float32).astype(np.float64)

    def hilo(v):
        hi = v.astype(_bf16)
        lo_ = (v - hi.astype(np.float64)).astype(_bf16)
        return hi, lo_

    bias_hi, bias_lo = hilo(bias)
    ct_hi, ct_lo = hilo(ct)
    bt = t.astype(_bf16)
    bt2 = (t * t).astype(_bf16)
    bp_hi = p.astype(_bf16)
    bp_lo = (p - bp_hi.astype(np.float64)).astype(_bf16)
    bc2 = _bf16(c2)
    pdum = _bf16(float(p.min()) - 1000.0)

    in_maps = []
    for c in range(NCORES):
        statm = np.zeros((7, NCH * P), dtype=_bf16)
        colvm = np.zeros((7, NCH * CH), dtype=_bf16)
        s = 0
        for b in assign[c]:
            r0 = P * b
            rows = slice(r0, r0 + P)
            for k in range(nch_b[b]):
                cstart = r0 - CH * (k + 1)
                statm[0, s * P:(s + 1) * P] = bias_hi[rows]
                statm[1, s * P:(s + 1) * P] = bias_lo[rows]
                statm[2, s * P:(s + 1) * P] = ct_hi[rows]
                statm[3, s * P:(s + 1) * P] = ct_lo[rows]
                statm[4, s * P:(s + 1) * P] = bc2
                statm[5, s * P:(s + 1) * P] = _bf16(1.0)
                statm[6, s * P:(s + 1) * P] = _bf16(1.0)
                cols = np.arange(cstart, cstart + CH)
                v = cols >= 0
                cc = np.clip(cols, 0, n - 1)
                sl = slice(s * CH, (s + 1) * CH)
                colvm[0, sl] = _bf16(1.0)
                colvm[1, sl] = _bf16(1.0)
                colvm[2, sl] = np.where(v, bt[cc], _bf16(0.0))
                colvm[3, sl] = colvm[2, sl]
                colvm[4, sl] = np.where(v, bt2[cc], _bf16(0.0))
                colvm[5, sl] = np.where(v, bp_hi[cc], pdum)
                colvm[6, sl] = np.where(v, bp_lo[cc], _bf16(0.0))
                s += 1
        # remaining chunks stay all-zero (z = 0 -> relu 0)
        # pack into the 3-lane table using the same mapping as build_nc
        CB = CH + P
        SZ, ENG, cl, cg3, ORDER, G3L = _group_plan(NCH)
        tblp = np.zeros((96, G3L * CB), dtype=_bf16)
        for s2 in range(NCH):
            lane, g3 = cl[s2], cg3[s2]
            rows = slice(32 * lane, 32 * lane + 7)
            tblp[rows, g3 * CB:g3 * CB + CH] = colvm[:, s2 * CH:(s2 + 1) * CH]
            tblp[rows, g3 * CB + CH:(g3 + 1) * CB] = statm[:, s2 * P:(s2 + 1) * P]
        in_maps.append({"tbl": tblp})
    return in_maps, host_total, NCH, n


def kernel(pred, target):
    pred = np.asarray(pred, dtype=np.float32)
    target = np.asarray(target, dtype=np.float32)
    in_maps, host_total, NCH, n = _prepare(pred, target)
    nc = _get_nc(NCH)
    run_bass_kernel_spmd(nc, in_maps, core_ids=list(range(NCORES)))
    res = run_bass_kernel_spmd(nc, in_maps, core_ids=list(range(NCORES)))
    total = host_total
    for r in res.results:
        total += float(np.asarray(r["acc"], np.float64).sum())
    K = n * (n - 1) // 2
    return np.float32(total / K)
